# revision 27
# baseline (speedup 1.0000x reference)
"""AffEncoder TRN2 Bass kernel (8-core data-parallel over batch).

The whole network folds into 4 temporal 1D-conv layers (host-side fold):
  L1: P[51,t]  -> F1[272,t]   (conv2d + A1 einsum + BN1)
  L2: F1       -> F2[96,t]    (body-part gather + conv2d + A2 einsum + BN2)
  L3: F2       -> F3[16,t]    (conv1d + BN3 + lrelu)
  L4: F3       -> OUT[64,t]   (conv1d + BN4 + lrelu)

On-chip: weights-stationary fp32r matmuls over 512-wide time tiles, with
tap pairs packed into the contraction dim via shifted row-block copies.
Each core handles 2 of the 16 batch sequences; outputs are concatenated.
"""
import sys
for p in ("/opt/trn_rl_repo", "/root/.axon_site/_ro/trn_rl_repo"):
    if p not in sys.path:
        sys.path.insert(0, p)

import numpy as np

BODY_PARTS = [[13, 14, 15, 16], [0, 3, 4], [1, 9, 10], [2, 11, 12], [5, 6], [7, 8]]
V1, V2 = 17, 6
F1C = 16
MAX_BP = 4
EPS = 1e-5
KPART = 5

N_CORES = 8
NSEQ = 2          # sequences per core
T = 2048
TT = 512          # time tile
NTILE = T // TT


# ---------------------------------------------------------------- folding

def fold_weights(inp):
    f8 = lambda x: np.asarray(x, dtype=np.float64)
    A1 = f8(inp["A1"]); A2 = f8(inp["A2"])
    w1 = f8(inp["w1"]); b1 = f8(inp["b1"])
    w2 = f8(inp["w2"]); b2 = f8(inp["b2"])
    w3 = f8(inp["w3"]); b3 = f8(inp["b3"])
    w4 = f8(inp["w4"]); b4 = f8(inp["b4"])
    g1, be1, m1, v1 = (f8(inp[k]) for k in ("g1", "be1", "m1", "v1"))
    g2, be2, m2, v2 = (f8(inp[k]) for k in ("g2", "be2", "m2", "v2"))
    g3, be3, m3, v3 = (f8(inp[k]) for k in ("g3", "be3", "m3", "v3"))
    g4, be4, m4, v4 = (f8(inp[k]) for k in ("g4", "be4", "m4", "v4"))

    s1 = g1 / np.sqrt(v1 + EPS)
    s2 = g2 / np.sqrt(v2 + EPS)
    s3 = g3 / np.sqrt(v3 + EPS)
    s4 = g4 / np.sqrt(v4 + EPS)

    w1r = w1.reshape(KPART, F1C, 3, 9, 5)  # [k, c, cin, tap, dvi]
    W1f = np.zeros((9, 51, 272))
    for dvi in range(5):
        for v in range(V1):
            u = v + dvi - 2
            if not (0 <= u < V1):
                continue
            contrib = np.einsum("kcit,kw->tciw", w1r[:, :, :, :, dvi], A1[:, v, :])
            for cin in range(3):
                W1f[:, u * 3 + cin, :] += contrib[:, :, cin, :].reshape(9, F1C * V1)
    W1f *= s1[None, None, :]
    bias_cw = np.einsum("kc,kw->cw", b1.reshape(KPART, F1C), A1.sum(axis=1)).reshape(F1C * V1)
    B1f = s1 * (bias_cw - m1) + be1

    joint_info = {}
    for p, parts in enumerate(BODY_PARTS):
        for l, j in enumerate(parts):
            joint_info[j] = (p, l, len(parts))
    w2r = w2.reshape(KPART, F1C, MAX_BP * F1C, 9, 3)
    W2f = np.zeros((9, 272, 96))
    for j in range(V1):
        p, l, L = joint_info[j]
        for cp in range(F1C):
            ci2 = cp * L + l
            ch1 = cp * V1 + j
            for dvi in range(3):
                vv2 = p - (dvi - 1)
                if not (0 <= vv2 < V2):
                    continue
                contrib = np.einsum("kct,kw->tcw", w2r[:, :, ci2, :, dvi], A2[:, vv2, :])
                W2f[:, ch1, :] += contrib.reshape(9, F1C * V2)
    W2f *= s2[None, None, :]
    bias2 = np.einsum("kc,kw->cw", b2.reshape(KPART, F1C), A2.sum(axis=1)).reshape(F1C * V2)
    B2f = s2 * (bias2 - m2) + be2

    W3f = (w3.transpose(2, 1, 0) * s3[None, None, :]).copy()
    B3f = s3 * (b3 - m3) + be3
    W4f = (w4.transpose(2, 1, 0) * s4[None, None, :]).copy()
    B4f = s4 * (b4 - m4) + be4

    return dict(
        W1f=W1f.astype(np.float32), B1f=B1f.astype(np.float32),
        W2f=W2f.astype(np.float32), B2f=B2f.astype(np.float32),
        W3f=W3f.astype(np.float32), B3f=B3f.astype(np.float32),
        W4f=W4f.astype(np.float32), B4f=B4f.astype(np.float32),
    )


# ---------------------------------------------------------------- wpack layout

class _Cols:
    """Static wpack column layouts (two packs: A loads first, B second)."""
    def __init__(self):
        # pack A: identity + L1 weights + L1 biases
        c = 0
        def take(n):
            nonlocal c
            s = c; c += n; return s
        self.w1p = [take(272) for _ in range(4)]   # rows 0..50 tap 2g, 64..114 tap 2g+1
        self.w1s = take(272)                       # rows 0..50 tap 8
        self.b1 = [take(1) for _ in range(3)]
        self.ncolA = c
        # pack B: everything else
        c = 0
        self.w2a = [take(96) for _ in range(9)]    # rows 0..127, F1 ch 0..127
        self.w2b = [take(96) for _ in range(9)]    # rows 0..127, F1 ch 128..255
        self.w2L0 = take(96)                       # rows 16g..16g+16 = W2f[g][256:272]
        self.w2s = take(96)                        # rows 0..15 tap 8 ch 256..271
        self.w3 = [take(16) for _ in range(5)]     # rows 0..95
        self.w4x = take(64)                        # rows 0..15 tap0, 32..47 tap-1, 64..79 tap+1
        self.b2 = take(1)
        self.b3 = take(1)
        self.b4 = take(1)
        self.ncolB = c

COLS = _Cols()

def build_wpack(fw):
    WA = np.zeros((128, COLS.ncolA), dtype=np.float32)
    for g in range(4):
        c0 = COLS.w1p[g]
        WA[0:51, c0:c0 + 272] = fw["W1f"][2 * g]
        WA[64:115, c0:c0 + 272] = fw["W1f"][2 * g + 1]
    WA[0:51, COLS.w1s:COLS.w1s + 272] = fw["W1f"][8]
    WA[0:96, COLS.b1[0]] = fw["B1f"][0:96]
    WA[0:96, COLS.b1[1]] = fw["B1f"][96:192]
    WA[0:80, COLS.b1[2]] = fw["B1f"][192:272]

    WB = np.zeros((128, COLS.ncolB), dtype=np.float32)
    for d in range(9):
        WB[0:128, COLS.w2a[d]:COLS.w2a[d] + 96] = fw["W2f"][d][0:128]
        WB[0:128, COLS.w2b[d]:COLS.w2b[d] + 96] = fw["W2f"][d][128:256]
    for g in range(8):
        WB[16 * g:16 * (g + 1), COLS.w2L0:COLS.w2L0 + 96] = fw["W2f"][g][256:272]
    WB[0:16, COLS.w2s:COLS.w2s + 96] = fw["W2f"][8][256:272]
    for e in range(5):
        WB[0:96, COLS.w3[e]:COLS.w3[e] + 16] = fw["W3f"][e]
    WB[0:16, COLS.w4x:COLS.w4x + 64] = fw["W4f"][1]
    WB[32:48, COLS.w4x:COLS.w4x + 64] = fw["W4f"][0]
    WB[64:80, COLS.w4x:COLS.w4x + 64] = fw["W4f"][2]
    WB[0:96, COLS.b2] = fw["B2f"]
    WB[0:16, COLS.b3] = fw["B3f"]
    WB[0:64, COLS.b4] = fw["B4f"]
    return WA, WB


# ---------------------------------------------------------------- device kernel

def build_nc():
    import concourse.bacc as bacc
    import concourse.mybir as mybir
    import concourse.tile as tile

    F32 = mybir.dt.float32
    F32R = mybir.dt.float32r
    AF = mybir.ActivationFunctionType

    nc = bacc.Bacc()
    poses_in = nc.declare_dram_parameter("poses", [NSEQ, T, 51], F32, isOutput=False)
    wpa_in = nc.declare_dram_parameter("wpackA", [128, COLS.ncolA], F32R, isOutput=False)
    wpb_in = nc.declare_dram_parameter("wpackB", [128, COLS.ncolB], F32R, isOutput=False)
    out_p = nc.declare_dram_parameter("out", [NSEQ, T, 64], F32, isOutput=True)

    with tile.TileContext(nc) as tc:
        with (
            tc.tile_pool(name="sb", bufs=1) as sb,
            tc.tile_pool(name="stgp", bufs=3) as stgp,
            tc.tile_pool(name="ps", bufs=4, space="PSUM") as ps,
            tc.tile_pool(name="psT", bufs=2, space="PSUM") as psT,
            tc.tile_pool(name="psO", bufs=2, space="PSUM") as psO,
        ):
            ident_t = sb.tile([128, 128], F32, name="ident", tag="ident")
            from concourse.masks import make_identity
            with tc.high_priority():
                make_identity(nc, ident_t[:])
            ident = ident_t[:]
            xstage_a = sb.tile([128, 8 * 51], F32, name="xstageA", tag="xstageA")
            xstage_b = sb.tile([128, 8 * 51], F32, name="xstageB", tag="xstageB")
            for h, xs in ((0, xstage_a), (1, xstage_b)):
                nc.sync.dma_start(
                    xs[:].rearrange("p (a f) -> p a f", f=51),
                    poses_in[0, 1024 * h:1024 * (h + 1), :]
                        .rearrange("(a p) f -> p a f", p=128),
                )
            wtA = sb.tile([128, COLS.ncolA], F32R, name="wtA", tag="wtA")
            wtB = sb.tile([128, COLS.ncolB], F32R, name="wtB", tag="wtB")
            nc.sync.dma_start(wtA[:], wpa_in[:])
            nc.sync.dma_start(wtB[:], wpb_in[:])

            X2 = []
            F1t0, F1t1, F1t2, L0, F2, F3x = [], [], [], [], [], []
            for n in range(NSEQ):
                X2.append(sb.tile([128, T + 8], F32R, name=f"X2_{n}", tag=f"X2_{n}"))
                F1t0.append(sb.tile([128, T + 8], F32R, name=f"F1t0_{n}", tag=f"F1t0_{n}"))
                F1t1.append(sb.tile([128, T + 8], F32R, name=f"F1t1_{n}", tag=f"F1t1_{n}"))
                F1t2.append(sb.tile([96, T + 8], F32R, name=f"F1t2_{n}", tag=f"F1t2_{n}"))
                L0.append(sb.tile([128, T + 8], F32R, name=f"L0_{n}", tag=f"L0_{n}"))
                F2.append(sb.tile([96, T + 4], F32R, name=f"F2_{n}", tag=f"F2_{n}"))
                F3x.append(sb.tile([96, T], F32R, name=f"F3x_{n}", tag=f"F3x_{n}"))

            for n in range(NSEQ):
                nc.gpsimd.memset(X2[n][32:64, :].bitcast(F32), 0.0)
                nc.gpsimd.memset(X2[n][96:128, :].bitcast(F32), 0.0)
                nc.gpsimd.memset(X2[n][0:128, 0:4].bitcast(F32), 0.0)
                nc.gpsimd.memset(X2[n][0:128, T + 4:T + 8].bitcast(F32), 0.0)
            for n in range(NSEQ):
                nc.gpsimd.memset(F1t2[n][:].bitcast(F32), 0.0)
                nc.gpsimd.memset(F3x[n][:].bitcast(F32), 0.0)
                for ft in (F1t0[n], F1t1[n]):
                    nc.gpsimd.memset(ft[:, 0:4].bitcast(F32), 0.0)
                    nc.gpsimd.memset(ft[:, T + 4:T + 8].bitcast(F32), 0.0)
                nc.gpsimd.memset(F2[n][:, 0:2].bitcast(F32), 0.0)
                nc.gpsimd.memset(F2[n][:, T + 2:T + 4].bitcast(F32), 0.0)

            # ---- input transposes: X rows 0..50, col c = x[c-4]
            for n in range(NSEQ):
                for h, xs in ((0, xstage_a), (1, xstage_b)):
                    if n > 0:
                        nc.sync.dma_start(
                            xs[:].rearrange("p (a f) -> p a f", f=51),
                            poses_in[n, 1024 * h:1024 * (h + 1), :]
                                .rearrange("(a p) f -> p a f", p=128),
                        )
                    for aa in range(8):
                        a = 8 * h + aa
                        pt = psT.tile([51, 128], F32, name="psT", tag="psT")
                        nc.tensor.transpose(pt[:], xs[:, 51 * aa:51 * aa + 51], ident)
                        nc.vector.tensor_copy(
                            X2[n][0:51, 4 + 128 * a:4 + 128 * (a + 1)], pt[:])
            for n in range(NSEQ):
                # B-block rows 64..114: B[.,c] = A[.,c+1]  (= x[c-3])
                half = (T + 7) // 2
                nc.vector.tensor_copy(X2[n][64:115, 0:half], X2[n][0:51, 1:half + 1])
                nc.scalar.activation(X2[n][64:115, half:T + 7],
                                     X2[n][0:51, half + 1:T + 8], AF.Copy)

            # ---- L1 ----
            mchunks = [(0, 96, 0), (96, 192, 1), (192, 272, 2)]
            for n in range(NSEQ):
                for i in range(NTILE):
                    t0 = TT * i
                    cs = slice(4 + t0, 4 + t0 + TT)
                    for (ms, me, mi) in mchunks:
                        mrows = me - ms
                        pp = ps.tile([mrows, TT], F32, name="ps", tag="ps")
                        for g in range(4):
                            c0 = COLS.w1p[g]
                            nc.tensor.matmul(
                                pp[:], wtA[0:128, c0 + ms:c0 + me],
                                X2[n][0:128, t0 + 2 * g:t0 + 2 * g + TT],
                                start=(g == 0), stop=False)
                        nc.tensor.matmul(
                            pp[:], wtA[0:128, COLS.w1s + ms:COLS.w1s + me],
                            X2[n][0:128, t0 + 8:t0 + 8 + TT],
                            start=False, stop=True)
                        bc = COLS.b1[mi]
                        if mi == 0:
                            nc.scalar.activation(
                                F1t0[n][0:96, cs], pp[:],
                                AF.Identity, bias=wtA[0:96, bc:bc + 1])
                        elif mi == 1:
                            nc.vector.tensor_scalar_add(
                                F1t0[n][96:128, cs], pp[0:32, :],
                                wtA[0:32, bc:bc + 1].bitcast(F32))
                            nc.vector.tensor_scalar_add(
                                F1t1[n][0:32, cs], pp[32:64, :],
                                wtA[32:64, bc:bc + 1].bitcast(F32))
                            nc.vector.tensor_scalar_add(
                                F1t1[n][32:64, cs], pp[64:96, :],
                                wtA[64:96, bc:bc + 1].bitcast(F32))
                        else:
                            nc.scalar.activation(
                                F1t1[n][64:128, cs], pp[0:64, :],
                                AF.Identity, bias=wtA[0:64, bc:bc + 1])
                            nc.scalar.activation(
                                F1t2[n][0:16, cs], pp[64:80, :],
                                AF.Identity, bias=wtA[64:80, bc:bc + 1])
                # L0 leftover im2col: L0[16g+r, c] = F1t2[r, c+g], taps g=0..7
                for g in range(8):
                    nc.sync.dma_start(
                        L0[n][16 * g:16 * (g + 1), 0:T + 8 - g],
                        F1t2[n][0:16, g:T + 8])

            # ---- L2 ----
            for n in range(NSEQ):
                for i in range(NTILE):
                    t0 = TT * i
                    pp = ps.tile([96, TT], F32, name="ps", tag="ps")
                    first = True
                    for d in range(9):
                        nc.tensor.matmul(
                            pp[:], wtB[0:128, COLS.w2a[d]:COLS.w2a[d] + 96],
                            F1t0[n][0:128, t0 + d:t0 + d + TT],
                            start=first, stop=False)
                        first = False
                    for d in range(9):
                        nc.tensor.matmul(
                            pp[:], wtB[0:128, COLS.w2b[d]:COLS.w2b[d] + 96],
                            F1t1[n][0:128, t0 + d:t0 + d + TT],
                            start=False, stop=False)
                    nc.tensor.matmul(
                        pp[:], wtB[0:128, COLS.w2L0:COLS.w2L0 + 96],
                        L0[n][0:128, t0:t0 + TT],
                        start=False, stop=False)
                    nc.tensor.matmul(
                        pp[:], wtB[0:96, COLS.w2s:COLS.w2s + 96],
                        F1t2[n][0:96, t0 + 8:t0 + 8 + TT],
                        start=False, stop=True)
                    nc.scalar.activation(
                        F2[n][0:96, 2 + t0:2 + t0 + TT], pp[:],
                        AF.Identity, bias=wtB[0:96, COLS.b2:COLS.b2 + 1])

            # ---- L3 ----  (F3x center rows 0..15 = f3[t] at col t)
            for n in range(NSEQ):
                for i in range(NTILE):
                    t0 = TT * i
                    pp = ps.tile([16, TT], F32, name="ps", tag="ps")
                    for e in range(5):
                        nc.tensor.matmul(
                            pp[:], wtB[0:96, COLS.w3[e]:COLS.w3[e] + 16],
                            F2[n][0:96, t0 + e:t0 + e + TT],
                            start=(e == 0), stop=(e == 4))
                    nc.scalar.activation(
                        F3x[n][0:16, t0:t0 + TT], pp[:],
                        AF.Lrelu, bias=wtB[0:16, COLS.b3:COLS.b3 + 1], alpha=0.01)
                # shifted copies: rows 32..47 = f3[t-1], rows 64..79 = f3[t+1]
                # (one on DVE, one on ACT so they run in parallel)
                nc.vector.tensor_copy(F3x[n][32:48, 1:T], F3x[n][0:16, 0:T - 1])
                nc.scalar.activation(F3x[n][64:80, 0:T - 1], F3x[n][0:16, 1:T],
                                     AF.Copy)

            # ---- L4 (all matmuls first), then output transposes + stores ----
            ots = {}
            for n in range(NSEQ):
                for i in range(NTILE):
                    t0 = TT * i
                    pp = ps.tile([64, TT], F32, name="ps", tag="ps")
                    nc.tensor.matmul(
                        pp[:], wtB[0:96, COLS.w4x:COLS.w4x + 64],
                        F3x[n][0:96, t0:t0 + TT],
                        start=True, stop=True)
                    ot = stgp.tile([128, TT], F32, name="ot", tag=f"ot{i % 2}",
                                   bufs=3)
                    nc.scalar.activation(
                        ot[0:64, :], pp[:],
                        AF.Lrelu, bias=wtB[0:64, COLS.b4:COLS.b4 + 1], alpha=0.01)
                    # rows 64..127 = chans of the NEXT 128-time chunk
                    nc.vector.tensor_copy(ot[64:128, 0:TT - 128], ot[0:64, 128:TT])
                    ots[(n, i)] = ot
            for n in range(NSEQ):
                for i in range(NTILE):
                    t0 = TT * i
                    ot = ots[(n, i)]
                    stg = stgp.tile([128, 256], F32, name="stg", tag="stg")
                    for j in range(2):
                        po = psO.tile([128, 128], F32, name="psO", tag="psO")
                        nc.tensor.transpose(
                            po[:], ot[0:128, 256 * j:256 * j + 128], ident)
                        nc.vector.tensor_copy(stg[:, 128 * j:128 * (j + 1)], po[:])
                    for hh in range(2):
                        nc.sync.dma_start(
                            out_p[n, t0 + 256 * hh:t0 + 256 * (hh + 1), :]
                                .rearrange("(j p) c -> p j c", p=128),
                            stg[:, 128 * hh:128 * (hh + 1)]
                                .rearrange("p (j c) -> p j c", c=64))

    nc.finalize()
    return nc


_NC = None

def _get_nc():
    global _NC
    if _NC is None:
        _NC = build_nc()
    return _NC


def run_device(inputs, trace=False, **trace_kwargs):
    """Returns (full_output, BassKernelResults)."""
    from concourse.bass_utils import run_bass_kernel_spmd

    fw = fold_weights(inputs)
    wpackA, wpackB = build_wpack(fw)
    poses = np.ascontiguousarray(np.asarray(inputs["poses"], dtype=np.float32))
    n_total = poses.shape[0]
    assert n_total == N_CORES * NSEQ and poses.shape[1] == T

    nc = _get_nc()
    in_maps = [
        dict(poses=poses[NSEQ * c:NSEQ * (c + 1)], wpackA=wpackA, wpackB=wpackB)
        for c in range(N_CORES)
    ]
    r = run_bass_kernel_spmd(nc, in_maps, list(range(N_CORES)), trace=trace,
                             **trace_kwargs)
    out = np.concatenate([r.results[c]["out"] for c in range(N_CORES)], axis=0)
    return out.astype(np.float32), r


def kernel(**inputs):
    out, _ = run_device(inputs)
    return out


# revision 28
# speedup vs baseline: 1.0255x; 1.0255x over previous
"""AffEncoder TRN2 Bass kernel (8-core data-parallel over batch).

The whole network folds into 4 temporal 1D-conv layers (host-side fold):
  L1: P[51,t]  -> F1[272,t]   (conv2d + A1 einsum + BN1)
  L2: F1       -> F2[96,t]    (body-part gather + conv2d + A2 einsum + BN2)
  L3: F2       -> F3[16,t]    (conv1d + BN3 + lrelu)
  L4: F3       -> OUT[64,t]   (conv1d + BN4 + lrelu)

On-chip: weights-stationary fp32r matmuls over 512-wide time tiles, with
tap pairs packed into the contraction dim via shifted row-block copies.
Each core handles 2 of the 16 batch sequences; outputs are concatenated.
"""
import sys
for p in ("/opt/trn_rl_repo", "/root/.axon_site/_ro/trn_rl_repo"):
    if p not in sys.path:
        sys.path.insert(0, p)

import numpy as np

BODY_PARTS = [[13, 14, 15, 16], [0, 3, 4], [1, 9, 10], [2, 11, 12], [5, 6], [7, 8]]
V1, V2 = 17, 6
F1C = 16
MAX_BP = 4
EPS = 1e-5
KPART = 5

N_CORES = 8
NSEQ = 2          # sequences per core
T = 2048
TT = 512          # time tile
NTILE = T // TT


# ---------------------------------------------------------------- folding

def fold_weights(inp):
    f8 = lambda x: np.asarray(x, dtype=np.float64)
    A1 = f8(inp["A1"]); A2 = f8(inp["A2"])
    w1 = f8(inp["w1"]); b1 = f8(inp["b1"])
    w2 = f8(inp["w2"]); b2 = f8(inp["b2"])
    w3 = f8(inp["w3"]); b3 = f8(inp["b3"])
    w4 = f8(inp["w4"]); b4 = f8(inp["b4"])
    g1, be1, m1, v1 = (f8(inp[k]) for k in ("g1", "be1", "m1", "v1"))
    g2, be2, m2, v2 = (f8(inp[k]) for k in ("g2", "be2", "m2", "v2"))
    g3, be3, m3, v3 = (f8(inp[k]) for k in ("g3", "be3", "m3", "v3"))
    g4, be4, m4, v4 = (f8(inp[k]) for k in ("g4", "be4", "m4", "v4"))

    s1 = g1 / np.sqrt(v1 + EPS)
    s2 = g2 / np.sqrt(v2 + EPS)
    s3 = g3 / np.sqrt(v3 + EPS)
    s4 = g4 / np.sqrt(v4 + EPS)

    w1r = w1.reshape(KPART, F1C, 3, 9, 5)  # [k, c, cin, tap, dvi]
    W1f = np.zeros((9, 51, 272))
    for dvi in range(5):
        for v in range(V1):
            u = v + dvi - 2
            if not (0 <= u < V1):
                continue
            contrib = np.einsum("kcit,kw->tciw", w1r[:, :, :, :, dvi], A1[:, v, :])
            for cin in range(3):
                W1f[:, u * 3 + cin, :] += contrib[:, :, cin, :].reshape(9, F1C * V1)
    W1f *= s1[None, None, :]
    bias_cw = np.einsum("kc,kw->cw", b1.reshape(KPART, F1C), A1.sum(axis=1)).reshape(F1C * V1)
    B1f = s1 * (bias_cw - m1) + be1

    joint_info = {}
    for p, parts in enumerate(BODY_PARTS):
        for l, j in enumerate(parts):
            joint_info[j] = (p, l, len(parts))
    w2r = w2.reshape(KPART, F1C, MAX_BP * F1C, 9, 3)
    W2f = np.zeros((9, 272, 96))
    for j in range(V1):
        p, l, L = joint_info[j]
        for cp in range(F1C):
            ci2 = cp * L + l
            ch1 = cp * V1 + j
            for dvi in range(3):
                vv2 = p - (dvi - 1)
                if not (0 <= vv2 < V2):
                    continue
                contrib = np.einsum("kct,kw->tcw", w2r[:, :, ci2, :, dvi], A2[:, vv2, :])
                W2f[:, ch1, :] += contrib.reshape(9, F1C * V2)
    W2f *= s2[None, None, :]
    bias2 = np.einsum("kc,kw->cw", b2.reshape(KPART, F1C), A2.sum(axis=1)).reshape(F1C * V2)
    B2f = s2 * (bias2 - m2) + be2

    W3f = (w3.transpose(2, 1, 0) * s3[None, None, :]).copy()
    B3f = s3 * (b3 - m3) + be3
    W4f = (w4.transpose(2, 1, 0) * s4[None, None, :]).copy()
    B4f = s4 * (b4 - m4) + be4

    return dict(
        W1f=W1f.astype(np.float32), B1f=B1f.astype(np.float32),
        W2f=W2f.astype(np.float32), B2f=B2f.astype(np.float32),
        W3f=W3f.astype(np.float32), B3f=B3f.astype(np.float32),
        W4f=W4f.astype(np.float32), B4f=B4f.astype(np.float32),
    )


# ---------------------------------------------------------------- wpack layout

class _Cols:
    """Static wpack column layouts (two packs: A loads first, B second)."""
    def __init__(self):
        # pack A: identity + L1 weights + L1 biases
        c = 0
        def take(n):
            nonlocal c
            s = c; c += n; return s
        self.w1p = [take(272) for _ in range(4)]   # rows 0..50 tap 2g, 64..114 tap 2g+1
        self.w1s = take(272)                       # rows 0..50 tap 8
        self.b1 = [take(1) for _ in range(3)]
        self.ncolA = c
        # pack B: everything else
        c = 0
        self.w2a = [take(96) for _ in range(9)]    # rows 0..127, F1 ch 0..127
        self.w2b = [take(96) for _ in range(9)]    # rows 0..127, F1 ch 128..255
        self.w2L0 = take(96)                       # rows 16g..16g+16 = W2f[g][256:272]
        self.w2s = take(96)                        # rows 0..15 tap 8 ch 256..271
        self.w3 = [take(16) for _ in range(5)]     # rows 0..95
        self.w4x = take(64)                        # rows 0..15 tap0, 32..47 tap-1, 64..79 tap+1
        self.b2 = take(1)
        self.b3 = take(1)
        self.b4 = take(1)
        self.ncolB = c

COLS = _Cols()

def build_wpack(fw):
    WA = np.zeros((128, COLS.ncolA), dtype=np.float32)
    for g in range(4):
        c0 = COLS.w1p[g]
        WA[0:51, c0:c0 + 272] = fw["W1f"][2 * g]
        WA[64:115, c0:c0 + 272] = fw["W1f"][2 * g + 1]
    WA[0:51, COLS.w1s:COLS.w1s + 272] = fw["W1f"][8]
    WA[0:96, COLS.b1[0]] = fw["B1f"][0:96]
    WA[0:96, COLS.b1[1]] = fw["B1f"][96:192]
    WA[0:80, COLS.b1[2]] = fw["B1f"][192:272]

    WB = np.zeros((128, COLS.ncolB), dtype=np.float32)
    for d in range(9):
        WB[0:128, COLS.w2a[d]:COLS.w2a[d] + 96] = fw["W2f"][d][0:128]
        WB[0:128, COLS.w2b[d]:COLS.w2b[d] + 96] = fw["W2f"][d][128:256]
    for g in range(8):
        WB[16 * g:16 * (g + 1), COLS.w2L0:COLS.w2L0 + 96] = fw["W2f"][g][256:272]
    WB[0:16, COLS.w2s:COLS.w2s + 96] = fw["W2f"][8][256:272]
    for e in range(5):
        WB[0:96, COLS.w3[e]:COLS.w3[e] + 16] = fw["W3f"][e]
    WB[0:16, COLS.w4x:COLS.w4x + 64] = fw["W4f"][1]
    WB[32:48, COLS.w4x:COLS.w4x + 64] = fw["W4f"][0]
    WB[64:80, COLS.w4x:COLS.w4x + 64] = fw["W4f"][2]
    WB[0:96, COLS.b2] = fw["B2f"]
    WB[0:16, COLS.b3] = fw["B3f"]
    WB[0:64, COLS.b4] = fw["B4f"]
    return WA, WB


# ---------------------------------------------------------------- device kernel

def build_nc():
    import concourse.bacc as bacc
    import concourse.mybir as mybir
    import concourse.tile as tile

    F32 = mybir.dt.float32
    F32R = mybir.dt.float32r
    AF = mybir.ActivationFunctionType

    nc = bacc.Bacc()
    poses_in = nc.declare_dram_parameter("poses", [NSEQ, T, 51], F32, isOutput=False)
    wpa_in = nc.declare_dram_parameter("wpackA", [128, COLS.ncolA], F32R, isOutput=False)
    wpb_in = nc.declare_dram_parameter("wpackB", [128, COLS.ncolB], F32R, isOutput=False)
    out_p = nc.declare_dram_parameter("out", [NSEQ, T, 64], F32, isOutput=True)

    with tile.TileContext(nc) as tc:
        with (
            tc.tile_pool(name="sb", bufs=1) as sb,
            tc.tile_pool(name="stgp", bufs=3) as stgp,
            tc.tile_pool(name="ps", bufs=4, space="PSUM") as ps,
            tc.tile_pool(name="psT", bufs=2, space="PSUM") as psT,
            tc.tile_pool(name="psO", bufs=2, space="PSUM") as psO,
        ):
            ident_t = sb.tile([128, 128], F32, name="ident", tag="ident")
            from concourse.masks import make_identity
            make_identity(nc, ident_t[:])
            ident = ident_t[:]
            xstage_a = sb.tile([128, 8 * 51], F32, name="xstageA", tag="xstageA")
            xstage_b = sb.tile([128, 8 * 51], F32, name="xstageB", tag="xstageB")
            for h, xs in ((0, xstage_a), (1, xstage_b)):
                nc.sync.dma_start(
                    xs[:].rearrange("p (a f) -> p a f", f=51),
                    poses_in[0, 1024 * h:1024 * (h + 1), :]
                        .rearrange("(a p) f -> p a f", p=128),
                )
            wtA = sb.tile([128, COLS.ncolA], F32R, name="wtA", tag="wtA")
            wtB = sb.tile([128, COLS.ncolB], F32R, name="wtB", tag="wtB")
            nc.sync.dma_start(wtA[:], wpa_in[:])
            nc.sync.dma_start(wtB[:], wpb_in[:])

            X2 = []
            F1t0, F1t1, F1t2, L0, F2, F3x = [], [], [], [], [], []
            for n in range(NSEQ):
                X2.append(sb.tile([128, T + 8], F32R, name=f"X2_{n}", tag=f"X2_{n}"))
                F1t0.append(sb.tile([128, T + 8], F32R, name=f"F1t0_{n}", tag=f"F1t0_{n}"))
                F1t1.append(sb.tile([128, T + 8], F32R, name=f"F1t1_{n}", tag=f"F1t1_{n}"))
                F1t2.append(sb.tile([96, T + 8], F32R, name=f"F1t2_{n}", tag=f"F1t2_{n}"))
                L0.append(sb.tile([128, T + 8], F32R, name=f"L0_{n}", tag=f"L0_{n}"))
                F2.append(sb.tile([96, T + 4], F32R, name=f"F2_{n}", tag=f"F2_{n}"))
                F3x.append(sb.tile([96, T], F32R, name=f"F3x_{n}", tag=f"F3x_{n}"))

            for n in range(NSEQ):
                nc.gpsimd.memset(X2[n][32:64, :].bitcast(F32), 0.0)
                nc.gpsimd.memset(X2[n][96:128, :].bitcast(F32), 0.0)
                nc.gpsimd.memset(X2[n][0:128, 0:4].bitcast(F32), 0.0)
                nc.gpsimd.memset(X2[n][0:128, T + 4:T + 8].bitcast(F32), 0.0)
            for n in range(NSEQ):
                nc.gpsimd.memset(F1t2[n][:].bitcast(F32), 0.0)
                nc.gpsimd.memset(F3x[n][:].bitcast(F32), 0.0)
                for ft in (F1t0[n], F1t1[n]):
                    nc.gpsimd.memset(ft[:, 0:4].bitcast(F32), 0.0)
                    nc.gpsimd.memset(ft[:, T + 4:T + 8].bitcast(F32), 0.0)
                nc.gpsimd.memset(F2[n][:, 0:2].bitcast(F32), 0.0)
                nc.gpsimd.memset(F2[n][:, T + 2:T + 4].bitcast(F32), 0.0)

            # ---- input transposes: X rows 0..50, col c = x[c-4]
            for n in range(NSEQ):
                for h, xs in ((0, xstage_a), (1, xstage_b)):
                    if n > 0:
                        nc.sync.dma_start(
                            xs[:].rearrange("p (a f) -> p a f", f=51),
                            poses_in[n, 1024 * h:1024 * (h + 1), :]
                                .rearrange("(a p) f -> p a f", p=128),
                        )
                    for aa in range(8):
                        a = 8 * h + aa
                        pt = psT.tile([51, 128], F32, name="psT", tag="psT")
                        nc.tensor.transpose(pt[:], xs[:, 51 * aa:51 * aa + 51], ident)
                        nc.vector.tensor_copy(
                            X2[n][0:51, 4 + 128 * a:4 + 128 * (a + 1)], pt[:])
            for n in range(NSEQ):
                # B-block rows 64..114: B[.,c] = A[.,c+1]  (= x[c-3])
                half = (T + 7) // 2
                nc.vector.tensor_copy(X2[n][64:115, 0:half], X2[n][0:51, 1:half + 1])
                nc.scalar.activation(X2[n][64:115, half:T + 7],
                                     X2[n][0:51, half + 1:T + 8], AF.Copy)

            # ---- L1 ----
            mchunks = [(0, 96, 0), (96, 192, 1), (192, 272, 2)]
            for n in range(NSEQ):
                for i in range(NTILE):
                    t0 = TT * i
                    cs = slice(4 + t0, 4 + t0 + TT)
                    for (ms, me, mi) in mchunks:
                        mrows = me - ms
                        pp = ps.tile([mrows, TT], F32, name="ps", tag="ps")
                        for g in range(4):
                            c0 = COLS.w1p[g]
                            nc.tensor.matmul(
                                pp[:], wtA[0:128, c0 + ms:c0 + me],
                                X2[n][0:128, t0 + 2 * g:t0 + 2 * g + TT],
                                start=(g == 0), stop=False)
                        nc.tensor.matmul(
                            pp[:], wtA[0:128, COLS.w1s + ms:COLS.w1s + me],
                            X2[n][0:128, t0 + 8:t0 + 8 + TT],
                            start=False, stop=True)
                        bc = COLS.b1[mi]
                        if mi == 0:
                            nc.scalar.activation(
                                F1t0[n][0:96, cs], pp[:],
                                AF.Identity, bias=wtA[0:96, bc:bc + 1])
                        elif mi == 1:
                            nc.vector.tensor_scalar_add(
                                F1t0[n][96:128, cs], pp[0:32, :],
                                wtA[0:32, bc:bc + 1].bitcast(F32))
                            nc.vector.tensor_scalar_add(
                                F1t1[n][0:32, cs], pp[32:64, :],
                                wtA[32:64, bc:bc + 1].bitcast(F32))
                            nc.vector.tensor_scalar_add(
                                F1t1[n][32:64, cs], pp[64:96, :],
                                wtA[64:96, bc:bc + 1].bitcast(F32))
                        else:
                            nc.scalar.activation(
                                F1t1[n][64:128, cs], pp[0:64, :],
                                AF.Identity, bias=wtA[0:64, bc:bc + 1])
                            nc.scalar.activation(
                                F1t2[n][0:16, cs], pp[64:80, :],
                                AF.Identity, bias=wtA[64:80, bc:bc + 1])
                # L0 leftover im2col: L0[16g+r, c] = F1t2[r, c+g], taps g=0..7
                for g in range(8):
                    nc.sync.dma_start(
                        L0[n][16 * g:16 * (g + 1), 0:T + 8 - g],
                        F1t2[n][0:16, g:T + 8])

            # ---- L2 ----
            for n in range(NSEQ):
                for i in range(NTILE):
                    t0 = TT * i
                    pp = ps.tile([96, TT], F32, name="ps", tag="ps")
                    first = True
                    for d in range(9):
                        nc.tensor.matmul(
                            pp[:], wtB[0:128, COLS.w2a[d]:COLS.w2a[d] + 96],
                            F1t0[n][0:128, t0 + d:t0 + d + TT],
                            start=first, stop=False)
                        first = False
                    for d in range(9):
                        nc.tensor.matmul(
                            pp[:], wtB[0:128, COLS.w2b[d]:COLS.w2b[d] + 96],
                            F1t1[n][0:128, t0 + d:t0 + d + TT],
                            start=False, stop=False)
                    nc.tensor.matmul(
                        pp[:], wtB[0:128, COLS.w2L0:COLS.w2L0 + 96],
                        L0[n][0:128, t0:t0 + TT],
                        start=False, stop=False)
                    nc.tensor.matmul(
                        pp[:], wtB[0:96, COLS.w2s:COLS.w2s + 96],
                        F1t2[n][0:96, t0 + 8:t0 + 8 + TT],
                        start=False, stop=True)
                    nc.scalar.activation(
                        F2[n][0:96, 2 + t0:2 + t0 + TT], pp[:],
                        AF.Identity, bias=wtB[0:96, COLS.b2:COLS.b2 + 1])

            # ---- L3 ----  (F3x center rows 0..15 = f3[t] at col t)
            for n in range(NSEQ):
                for i in range(NTILE):
                    t0 = TT * i
                    pp = ps.tile([16, TT], F32, name="ps", tag="ps")
                    for e in range(5):
                        nc.tensor.matmul(
                            pp[:], wtB[0:96, COLS.w3[e]:COLS.w3[e] + 16],
                            F2[n][0:96, t0 + e:t0 + e + TT],
                            start=(e == 0), stop=(e == 4))
                    nc.scalar.activation(
                        F3x[n][0:16, t0:t0 + TT], pp[:],
                        AF.Lrelu, bias=wtB[0:16, COLS.b3:COLS.b3 + 1], alpha=0.01)
                # shifted copies: rows 32..47 = f3[t-1], rows 64..79 = f3[t+1]
                # (one on DVE, one on ACT so they run in parallel)
                nc.vector.tensor_copy(F3x[n][32:48, 1:T], F3x[n][0:16, 0:T - 1])
                nc.scalar.activation(F3x[n][64:80, 0:T - 1], F3x[n][0:16, 1:T],
                                     AF.Copy)

            # ---- L4 (all matmuls first), then output transposes + stores ----
            ots = {}
            for n in range(NSEQ):
                for i in range(NTILE):
                    t0 = TT * i
                    pp = ps.tile([64, TT], F32, name="ps", tag="ps")
                    nc.tensor.matmul(
                        pp[:], wtB[0:96, COLS.w4x:COLS.w4x + 64],
                        F3x[n][0:96, t0:t0 + TT],
                        start=True, stop=True)
                    ot = stgp.tile([128, TT], F32, name="ot", tag=f"ot{i % 2}",
                                   bufs=3)
                    nc.scalar.activation(
                        ot[0:64, :], pp[:],
                        AF.Lrelu, bias=wtB[0:64, COLS.b4:COLS.b4 + 1], alpha=0.01)
                    # rows 64..127 = chans of the NEXT 128-time chunk
                    nc.vector.tensor_copy(ot[64:128, 0:TT - 128], ot[0:64, 128:TT])
                    ots[(n, i)] = ot
            for n in range(NSEQ):
                for i in range(NTILE):
                    t0 = TT * i
                    ot = ots[(n, i)]
                    stg = stgp.tile([128, 256], F32, name="stg", tag="stg")
                    for j in range(2):
                        po = psO.tile([128, 128], F32, name="psO", tag="psO")
                        nc.tensor.transpose(
                            po[:], ot[0:128, 256 * j:256 * j + 128], ident)
                        nc.vector.tensor_copy(stg[:, 128 * j:128 * (j + 1)], po[:])
                    nc.sync.dma_start(
                        out_p[n, t0:t0 + TT, :].rearrange("(j p) c -> p j c", p=128),
                        stg[:].rearrange("p (j c) -> p j c", c=64))

    nc.finalize()
    return nc


_NC = None

def _get_nc():
    global _NC
    if _NC is None:
        _NC = build_nc()
    return _NC


def run_device(inputs, trace=False, **trace_kwargs):
    """Returns (full_output, BassKernelResults)."""
    from concourse.bass_utils import run_bass_kernel_spmd

    fw = fold_weights(inputs)
    wpackA, wpackB = build_wpack(fw)
    poses = np.ascontiguousarray(np.asarray(inputs["poses"], dtype=np.float32))
    n_total = poses.shape[0]
    assert n_total == N_CORES * NSEQ and poses.shape[1] == T

    nc = _get_nc()
    in_maps = [
        dict(poses=poses[NSEQ * c:NSEQ * (c + 1)], wpackA=wpackA, wpackB=wpackB)
        for c in range(N_CORES)
    ]
    r = run_bass_kernel_spmd(nc, in_maps, list(range(N_CORES)), trace=trace,
                             **trace_kwargs)
    out = np.concatenate([r.results[c]["out"] for c in range(N_CORES)], axis=0)
    return out.astype(np.float32), r


def kernel(**inputs):
    out, _ = run_device(inputs)
    return out
